# revision 38
# baseline (speedup 1.0000x reference)
"""AdaptiveSAGE GNN message-passing kernel for 8 TRN2 NeuronCores.

Sharding: by DESTINATION node across 8 cores (6250 dst nodes per core) so
each core exclusively owns its output slice -> no collective needed.  The
host does data movement / planning only: edge sorting, padding, index
packing, window packing, and materialization of each core's per-edge
source-feature stream (a gather = pure data movement; h rows are laid out
in the order the core's edge tiles consume them, so the device streams them
sequentially at full DMA bandwidth).  All FLOPs (coefficient products,
message scaling, segment-sum, mean, MLP, relu) run on device.

ONE multi-tile scalar_tensor_tensor builds ALL TW one-hots of a window:
out[p,j,s] = (iota[s] == slotw[p]) * coeff[p,t0+j], with the coefficients
entering as a [P,TW,1]->[P,TW,WINW] stride-0 broadcast and the window's
slot vector as the single AP scalar (~59ns/tile vs 150 for per-tile ops --
the 58c init + 58c scalar-load amortize over TW tiles).  This requires the
slot vector to be constant across a window's tiles, so the host packs each
dst node onto ceil(deg/TW) OWNED PARTITIONS of one window (greedy
largest-fit, <=WINW dsts and <=128 partitions per window); the deg-mod-TW
grid residue costs ~14% tile padding, a net win since the kernel is no
longer DVE-bound (DMA ~87% busy, PE LDWEIGHTS co-critical; WINW=32 is the
measured optimum -- wider windows raise DVE faster than they cut bytes,
and both 2MB and 4MB chunk sizes measured worse than this config).
GROUP windows share one PSUM bank region-wise so evacuation (ACT copy +
N=GROUP*WINW MLP matmul + relu + out-DMA) is amortized over 512 columns.

Device pipeline per core:
  - stream hg (pre-laid-out h[src] rows, bf16) in ramped chunks on the
    sync (SP) HWDGE ring; meta/W/b/outputs use the scalar (ACT) ring or
    gpsimd SWDGE so the hg stream never queues behind them.
  - DVE builds a scaled one-hot per 128-edge tile in one fused op:
        oh[e, slot] = (iota[slot] == slot_e) * coeff_e,
        coeff_e = alpha[idx_e] * edge_weight_e * (1/deg[dst_e])  (mean folded)
  - TensorE: psum[dim, slot] += hg[e, dim]^T-contract oh[e, slot] (segment sum)
  - per window: MLP psum2[j, slot] = W^T @ cast_bf16(psum);
    relu(+b) into a multi-window batch tile; DMA out per batch.
Host scatters out[128, NWIN*WINW] per core back to z[50000, 128] via the
window/slot -> node map.
"""

import sys

if "/opt/trn_rl_repo" not in sys.path:
    sys.path.insert(0, "/opt/trn_rl_repo")

import numpy as np
import ml_dtypes

import concourse.bass as bass
import concourse.bacc as bacc
import concourse.mybir as mybir
import concourse.tile as tile
from concourse.bass_utils import run_bass_kernel_spmd

N_NODES = 50000
DIM = 128
NCORES = 8
NPC = N_NODES // NCORES          # 6250 dst nodes per core
WINW = 32                        # dst-window width (one-hot/psum free dim)
TW = 5                           # tiles per window (fixed; one STT builds all 5)
CHUNK_TILES = 64                 # steady-state tiles per hg stream chunk (2 MB)
SEG_BOUNDS = (30, 190, 640)      # coeff segment cuts (multiples of TW):
                                 # [0,30) on DVE, rest gpsimd/deferred
GROUP = 16                       # windows per PSUM bank / evacuation batch
P = 128

f32 = mybir.dt.float32
bf16 = mybir.dt.bfloat16


def _pack_partitions(deg):
    """Assign each dst node ceil(deg/TW) partitions within some window so the
    window's slot vector is constant across its TW tiles (one multi-tile STT
    builds all TW one-hots).  Windows take <=WINW dsts and <=128 partitions.
    Greedy largest-fit keeps partitions ~full.  Returns per-node (win, slot,
    p0) and the number of windows."""
    q = np.maximum((deg + TW - 1) // TW, 1).astype(np.int64)  # partitions/dst
    order = np.argsort(-deg, kind="stable")
    # pools by q, each a stack (desc degree -> pop from front via index)
    pools = {}
    for idx in order:
        pools.setdefault(int(q[idx]), []).append(int(idx))
    ptrs = {k: 0 for k in pools}
    qs_desc = sorted(pools.keys(), reverse=True)

    win_of = np.full(NPC, -1, np.int64)
    slot_of = np.zeros(NPC, np.int64)
    p0_of = np.zeros(NPC, np.int64)
    n_left = NPC
    w = 0
    while n_left > 0:
        parts = 0
        slots = 0
        while slots < WINW and parts < 128:
            qv = None
            for qc in qs_desc:
                if ptrs[qc] < len(pools[qc]) and parts + qc <= 128:
                    qv = qc
                    break
            if qv is None:
                break
            idx = pools[qv][ptrs[qv]]
            ptrs[qv] += 1
            win_of[idx] = w
            slot_of[idx] = slots
            p0_of[idx] = parts
            parts += qv
            slots += 1
            n_left -= 1
        w += 1
    return win_of, slot_of, p0_of, w


def _preprocess(h, alpha, edge_weight, W, b, node_id, edge_src, edge_dst):
    """Host-side planning: sort/pad edges, pack device images. Data movement only."""
    src = np.asarray(edge_src).astype(np.int64)
    dst = np.asarray(edge_dst).astype(np.int64)
    node_id = np.asarray(node_id).astype(np.int64)
    alpha = np.asarray(alpha, dtype=np.float32)
    ew = np.asarray(edge_weight, dtype=np.float32)
    E = src.shape[0]
    gene_num = alpha.shape[0] - 2

    src_id = node_id[src]
    dst_id = node_id[dst]
    gi = np.full(E, gene_num + 1, np.int64)
    gi = np.where((src_id >= 0) & (dst_id < 0), src_id, gi)
    gi = np.where((dst_id >= 0) & (src_id < 0), dst_id, gi)
    gi = np.where((dst_id >= 0) & (src_id >= 0), gene_num, gi)
    a_e = alpha[gi]                                   # gather (data movement)

    deg = np.bincount(dst, minlength=N_NODES).astype(np.float32)
    r_e = 1.0 / np.maximum(deg[dst], 1.0)             # mean norm (metadata)

    core = dst // NPC
    ldst = dst - core * NPC

    # per-core partition-ownership packing (planning only)
    degc = np.bincount(dst, minlength=N_NODES).astype(np.int64).reshape(NCORES, NPC)
    win_of = np.zeros((NCORES, NPC), np.int64)
    slot_of = np.zeros((NCORES, NPC), np.int64)
    p0_of = np.zeros((NCORES, NPC), np.int64)
    nw = 0
    for c in range(NCORES):
        win_of[c], slot_of[c], p0_of[c], w_c = _pack_partitions(degc[c])
        nw = max(nw, w_c)
    NWIN = nw
    TT = NWIN * TW
    EP = TT * P

    # per-edge placement: edges of dst d fill its q_d x TW grid row-major
    eorder = np.argsort(core * NPC + ldst, kind="stable")   # sort by (core,dst)
    dkey = (core * NPC + ldst)[eorder]
    first = np.ones(E, bool)
    first[1:] = dkey[1:] != dkey[:-1]
    gstart = np.where(first)[0]
    gsid = np.cumsum(first) - 1
    rank = np.arange(E, dtype=np.int64) - gstart[gsid]      # rank within dst

    ec = core[eorder]
    el = ldst[eorder]
    part = p0_of[ec, el] + rank // TW
    tl = win_of[ec, el] * TW + rank % TW
    pos = tl * P + part                                      # within core image

    gidx_p = np.zeros((NCORES, EP), np.int32)
    a_p = np.zeros((NCORES, EP), np.float32)
    w_p = np.zeros((NCORES, EP), np.float32)
    cnt_p = np.zeros((NCORES, EP), np.float32)
    gidx_p[ec, pos] = src[eorder].astype(np.int32)
    a_p[ec, pos] = a_e[eorder]
    w_p[ec, pos] = ew[eorder]
    cnt_p[ec, pos] = r_e[eorder]
    # per-tile slot image kept for the test-harness emulator only
    slot_p = np.zeros((NCORES, EP), np.float32)
    slot_p[ec, pos] = slot_of[ec, el].astype(np.float32)

    # per-(partition, window) slot vector for the device (constant over TW tiles)
    slotw = np.zeros((NCORES, P, NWIN), np.float32)
    for c in range(NCORES):
        owned = win_of[c] >= 0
        d = np.where(owned)[0]
        for di in d:
            q = max((degc[c][di] + TW - 1) // TW, 1)
            slotw[c, p0_of[c][di]:p0_of[c][di] + q, win_of[c][di]] = slot_of[c][di]

    # images: edge pos = t*128 + p  ->  [p, t]
    def img(x):
        return np.ascontiguousarray(x.reshape(NCORES, TT, P).transpose(0, 2, 1))

    a_i, w_i, cnt_i, slot_i = img(a_p), img(w_p), img(cnt_p), img(slot_p)
    # packed per-segment meta images: blocks [cnt | a | w] so ONE DMA brings a
    # whole segment; meta0 additionally carries [slotw | bias]
    cuts = [0] + [min(b, TT) for b in SEG_BOUNDS] + [TT]
    segs = [(lo, hi) for lo, hi in zip(cuts, cuts[1:]) if hi > lo]

    def meta_img(lo, hi):
        return np.ascontiguousarray(np.concatenate(
            [cnt_i[:, :, lo:hi], a_i[:, :, lo:hi], w_i[:, :, lo:hi]], axis=2))

    metas = [meta_img(lo, hi) for lo, hi in segs]
    b_img = np.broadcast_to(
        np.asarray(b, np.float32).reshape(1, DIM, 1), (NCORES, DIM, 1))
    metas[0] = np.ascontiguousarray(
        np.concatenate([metas[0], slotw, b_img], axis=2))

    h_bf = np.asarray(h, np.float32).astype(ml_dtypes.bfloat16)
    # per-core source-feature stream, laid out exactly as consumed:
    # [128 partitions, TT tiles, DIM] with edge (t, p) at [p, t, :]
    hg_img = np.ascontiguousarray(
        h_bf[gidx_p.reshape(NCORES, TT, P)].transpose(0, 2, 1, 3))

    # out column (w*WINW + s) -> global node id (or -1)
    outmap = np.full((NCORES, NWIN * WINW), -1, np.int64)
    for c in range(NCORES):
        cols = win_of[c] * WINW + slot_of[c]
        outmap[c, cols] = c * NPC + np.arange(NPC)

    T = np.full(NWIN, TW, np.int64)
    tile_off = np.arange(NWIN, dtype=np.int64) * TW
    plan = dict(
        T=T, TT=TT, EP=EP, NWIN=NWIN, tile_off=tile_off, segs=segs,
        hg_img=hg_img, metas=metas, outmap=outmap,
        # kept for test harness emulation compatibility
        slot_img=slot_i, a_img=a_i, w_img=w_i, cnt_img=cnt_i, idx_img=img(gidx_p),
        wt_bf=np.ascontiguousarray(np.asarray(W, np.float32).T).astype(ml_dtypes.bfloat16),
        b_col=np.ascontiguousarray(np.asarray(b, np.float32).reshape(DIM, 1)),
    )
    return plan


def _reassemble(plan, outs):
    """outs: per-core [128, NWIN*WINW] arrays -> z [N_NODES, DIM]."""
    z = np.empty((N_NODES, DIM), np.float32)
    outmap = plan["outmap"]
    for c in range(NCORES):
        valid = outmap[c] >= 0
        z[outmap[c][valid]] = np.asarray(outs[c])[:, valid].T
    return z


def _build(plan):
    """Build the (SPMD-identical) Bass graph from the static plan."""
    TT = plan["TT"]
    NWIN = plan["NWIN"]
    segs = plan["segs"]

    nc = bacc.Bacc("TRN2", target_bir_lowering=False, debug=False,
                   num_swdge_queues=4)
    hg_d = nc.dram_tensor("hgimg", [P, TT, DIM], bf16, kind="ExternalInput")
    meta_d = [nc.dram_tensor(f"meta{i}",
                             [P, 3 * (hi - lo) + (NWIN + 1 if i == 0 else 0)],
                             f32, kind="ExternalInput")
              for i, (lo, hi) in enumerate(segs)]
    wt_d = nc.dram_tensor("wt", [DIM, DIM], bf16, kind="ExternalInput")
    out_d = nc.dram_tensor("out", [P, NWIN * WINW], f32, kind="ExternalOutput")

    with tile.TileContext(nc) as tc:
        with (
            tc.tile_pool(name="const", bufs=1) as cpool,
            tc.tile_pool(name="gather", bufs=5) as gpool,
            tc.tile_pool(name="oh", bufs=64) as ohpool,
            tc.tile_pool(name="mlp", bufs=3) as mpool,
            tc.tile_pool(name="nbw", bufs=4) as nbwpool,
            tc.tile_pool(name="zb", bufs=3) as zpool,
            tc.tile_pool(name="psum", bufs=2, space="PSUM") as pspool,
            tc.tile_pool(name="psw", bufs=3, space="PSUM") as pswpool,
            tc.tile_pool(name="psum2", bufs=2, space="PSUM") as ps2pool,
        ):
            iota_f = cpool.tile([P, WINW], f32, tag="iotaf")
            nc.gpsimd.iota(iota_f[:], pattern=[[1, WINW]], base=0,
                           channel_multiplier=0,
                           allow_small_or_imprecise_dtypes=True)
            iotar = cpool.tile([P, TW * WINW], bf16, tag="iotar")
            for j in range(TW):
                nc.vector.tensor_copy(out=iotar[:, j * WINW:(j + 1) * WINW],
                                      in_=iota_f[:])
            # I32[p, s] = (s == p): top-left WINW x WINW identity, used to
            # transpose each window's [WINW, DIM] psum into the group bank
            pidx = cpool.tile([P, 1], f32, tag="pidx")
            nc.gpsimd.iota(pidx[:], pattern=[[1, 1]], base=0,
                           channel_multiplier=1,
                           allow_small_or_imprecise_dtypes=True)
            i32 = cpool.tile([P, WINW], bf16, tag="i32")
            nc.vector.tensor_scalar(out=i32[:], in0=iotar[:, :WINW],
                                    scalar1=pidx[:, :1], scalar2=None,
                                    op0=mybir.AluOpType.is_equal)

            # coeff = a*w*(1/cnt) per segment.  Segment 0 on DVE via the
            # scalar (ACT) HWDGE ring (fast pipeline head); segments 1-2 on
            # gpsimd (own SWDGE queue) so the DVE one-hot stream and the
            # sync ring (hg chunks) stay clear.
            meta_sb, coeff_sb = [], []
            for i, (lo, hi) in enumerate(segs):
                n = hi - lo
                m = cpool.tile([P, 3 * n + (NWIN + 1 if i == 0 else 0)], f32,
                               tag=f"meta{i}", name="meta")
                c_ = cpool.tile([P, n], f32, tag=f"coeff{i}", name="coeff")
                meta_sb.append(m)
                coeff_sb.append(c_)

            def emit_seg(i):
                # segs 0-1: ACT-ring DMA + DVE products (fast pipeline head);
                # later segs: gpsimd products, last seg's DMA deferred into
                # the ACT engine's program order (emitted at a later window)
                # to keep its bytes out of the congested early HBM window
                lo, hi = segs[i]
                n = hi - lo
                m, c_ = meta_sb[i], coeff_sb[i]
                if i == 0:
                    nc.scalar.dma_start(m[:], meta_d[i].ap()[:])
                    eng = nc.vector
                elif i >= 2:
                    # deferred: DMA rides the ACT ring at a later window so
                    # its bytes stay out of the HBM-saturated early phase
                    nc.scalar.dma_start(m[:], meta_d[i].ap()[:])
                    eng = nc.gpsimd
                else:
                    nc.gpsimd.dma_start(m[:], meta_d[i].ap()[:])
                    eng = nc.gpsimd
                eng.tensor_tensor(out=c_[:], in0=m[:, n:2 * n],
                                  in1=m[:, 2 * n:3 * n],
                                  op=mybir.AluOpType.mult)
                eng.tensor_tensor(out=c_[:], in0=c_[:], in1=m[:, 0:n],
                                  op=mybir.AluOpType.mult)

            emit_seg(0)

            def seg_of(t):
                for i, (lo, hi) in enumerate(segs):
                    if t < hi:
                        return i, t - lo
                raise AssertionError

            n0 = segs[0][1] - segs[0][0]

            def slotw_ap(w):
                return meta_sb[0][:, 3 * n0 + w: 3 * n0 + w + 1]

            def coeff_run(t0):
                # [P, TW] view of the coeff columns for tiles t0..t0+TW-1
                # (SEG_BOUNDS are multiples of TW so a window never straddles)
                i, k = seg_of(t0)
                return coeff_sb[i][:, k: k + TW]

            wt_sb = cpool.tile([DIM, DIM], bf16, tag="wt")
            nc.sync.dma_start(wt_sb[:], wt_d.ap()[:])
            b_sb = meta_sb[0]

            # ramped chunk plan: small head chunks land fast even while the
            # meta/weight transfers share the SDMA engines, then 64s
            chunks = []
            t0c = 0
            for first in (4, 8, 16, 32):
                if t0c < TT:
                    nt = min(first, TT - t0c)
                    chunks.append((t0c, nt))
                    t0c += nt
            while t0c < TT:
                nt = min(CHUNK_TILES, TT - t0c)
                chunks.append((t0c, nt))
                t0c += nt
            tile2chunk = {}
            for ci, (c0, nt) in enumerate(chunks):
                for k in range(nt):
                    tile2chunk[c0 + k] = (ci, k)

            stream_tiles = {}

            def ensure_streamed(ci):
                if ci in stream_tiles:
                    return stream_tiles[ci]
                c0, nt = chunks[ci]
                hg = gpool.tile([P, CHUNK_TILES, DIM], bf16, tag="hg", name="hg")
                nc.sync.dma_start(hg[:, :nt, :], hg_d.ap()[:, c0:c0 + nt, :])
                stream_tiles[ci] = hg
                return hg

            if len(segs) > 1:
                emit_seg(1)
            # segs 2+ emitted inside the window loop (window 15, 40, ...)
            defer_at = {15 + 25 * (i - 2): i for i in range(2, len(segs))}

            # GROUP windows share one PSUM bank (GROUP*WINW <= 512 f32);
            # start=True only on the group's first matmul: it clears the
            # whole bank's has_written bits, each later window's first MM
            # then overwrites its own (cleared) column region and subsequent
            # MMs accumulate -- one ACT copy + one N=GROUP*WINW MLP matmul
            # + one relu + one out-DMA per GROUP instead of per window.
            psumG = None
            for w in range(NWIN):
                if w in defer_at:
                    emit_seg(defer_at.pop(w))
                t0 = w * TW
                g = w % GROUP
                if g == 0:
                    psumG = pspool.tile([P, GROUP * WINW], f32, tag="ps",
                                        name="psum")
                last_of_group = (g == GROUP - 1) or (w == NWIN - 1)
                # ONE multi-tile STT builds all tw one-hots of the window:
                # out[p, j, s] = (iota[s] == slotw[p]) * coeff[p, t0+j]
                oh5 = ohpool.tile([P, TW * WINW], bf16, tag="oh", name="oh")
                nc.vector.scalar_tensor_tensor(
                    out=oh5[:].rearrange("p (k s) -> p k s", k=TW),
                    in0=iotar[:].rearrange("p (k s) -> p k s", k=TW),
                    scalar=slotw_ap(w),
                    in1=coeff_run(t0).unsqueeze(2).broadcast_to([P, TW, WINW]),
                    op0=mybir.AluOpType.is_equal,
                    op1=mybir.AluOpType.mult,
                )
                # flipped segment-sum: oh is the 32-col STATIONARY operand
                # (27ns LDW, hidden under the 53ns N=128 hg stream) and hg
                # streams as the moving operand -> psw[slot, dim]
                psw = pswpool.tile([WINW, DIM], f32, tag="psw", name="psw")
                for j in range(TW):
                    t = t0 + j
                    ci, kk = tile2chunk[t]
                    hg = ensure_streamed(ci)
                    nc.tensor.matmul(
                        psw[:], oh5[:, j * WINW:(j + 1) * WINW], hg[:, kk, :],
                        start=(j == 0), stop=(j == TW - 1),
                    )
                # transpose [WINW, DIM] -> [DIM, WINW] into the group bank:
                # psumG[:, g] = nb^T via matmul(lhsT=nb, rhs=I32-top)
                nb = nbwpool.tile([WINW, DIM], bf16, tag="nbw", name="nbw")
                nc.scalar.copy(nb[:], psw[:])
                nc.tensor.matmul(
                    psumG[:, g * WINW:(g + 1) * WINW], nb[:], i32[:WINW, :],
                    start=True, stop=True,
                )
                if last_of_group:
                    gw = (g + 1) * WINW
                    w0 = w - g
                    nbf = mpool.tile([P, GROUP * WINW], bf16, tag="nbf",
                                     name="nbf")
                    nc.scalar.copy(nbf[:, :gw], psumG[:, :gw])
                    psum2 = ps2pool.tile([P, GROUP * WINW], f32, tag="ps2",
                                         name="psum2")
                    nc.tensor.matmul(psum2[:, :gw], wt_sb[:], nbf[:, :gw],
                                     start=True, stop=True)
                    zb = zpool.tile([P, GROUP * WINW], f32, tag="zb",
                                    name="zbat")
                    nc.scalar.activation(zb[:, :gw], psum2[:, :gw],
                                         mybir.ActivationFunctionType.Relu,
                                         bias=b_sb[:, 3 * n0 + NWIN:
                                                   3 * n0 + NWIN + 1])
                    nc.scalar.dma_start(
                        out_d.ap()[:, w0 * WINW: w0 * WINW + gw],
                        zb[:, :gw])

    nc.compile()
    return nc


def _in_maps(plan):
    maps = []
    for c in range(NCORES):
        m = {
            "hgimg": plan["hg_img"][c],
            "wt": plan["wt_bf"],
        }
        for i in range(len(plan["segs"])):
            m[f"meta{i}"] = plan["metas"][i][c]
        maps.append(m)
    return maps


_NC_CACHE = {}


def _get_nc(plan):
    key = (plan["TT"], tuple(plan["T"]))
    if key not in _NC_CACHE:
        _NC_CACHE[key] = _build(plan)
    return _NC_CACHE[key]


def kernel(**inputs):
    plan = _preprocess(**{k: np.asarray(v) for k, v in inputs.items()})
    nc = _get_nc(plan)
    res = run_bass_kernel_spmd(nc, _in_maps(plan), core_ids=list(range(NCORES)))
    return _reassemble(plan, [res.results[c]["out"] for c in range(NCORES)])


# revision 40
# speedup vs baseline: 1.2462x; 1.2462x over previous
"""AdaptiveSAGE GNN message-passing kernel for 8 TRN2 NeuronCores.

Sharding: by DESTINATION node across 8 cores (6250 dst nodes per core) so
each core exclusively owns its output slice -> no collective needed.  The
host does data movement / planning only: edge sorting, padding, index
packing, window packing, and materialization of each core's per-edge
source-feature stream (a gather = pure data movement; h rows are laid out
in the order the core's edge tiles consume them, so the device streams them
sequentially at full DMA bandwidth).  All FLOPs (coefficient products,
message scaling, segment-sum, mean, MLP, relu) run on device.

ONE multi-tile scalar_tensor_tensor builds ALL TW one-hots of a window:
out[p,j,s] = (iota[s] == slotw[p]) * coeff[p,t0+j], with the coefficients
entering as a [P,TW,1]->[P,TW,WINW] stride-0 broadcast and the window's
slot vector as the single AP scalar (~59ns/tile vs 150 for per-tile ops --
the 58c init + 58c scalar-load amortize over TW tiles).  This requires the
slot vector to be constant across a window's tiles, so the host packs each
dst node onto ceil(deg/TW) OWNED PARTITIONS of one window (greedy
largest-fit, <=WINW dsts and <=128 partitions per window); the deg-mod-TW
grid residue costs ~14% tile padding, a net win since the kernel is no
longer DVE-bound (DMA ~87% busy, PE LDWEIGHTS co-critical; WINW=32 is the
measured optimum -- wider windows raise DVE faster than they cut bytes,
and both 2MB and 4MB chunk sizes measured worse than this config).
GROUP windows share one PSUM bank region-wise so evacuation (ACT copy +
N=GROUP*WINW MLP matmul + relu + out-DMA) is amortized over 512 columns.

Device pipeline per core:
  - stream hg (pre-laid-out h[src] rows, bf16) in ramped chunks on the
    sync (SP) HWDGE ring; meta/W/b/outputs use the scalar (ACT) ring or
    gpsimd SWDGE so the hg stream never queues behind them.
  - DVE builds a scaled one-hot per 128-edge tile in one fused op:
        oh[e, slot] = (iota[slot] == slot_e) * coeff_e,
        coeff_e = alpha[idx_e] * edge_weight_e * (1/deg[dst_e])  (mean folded)
  - TensorE: psum[dim, slot] += hg[e, dim]^T-contract oh[e, slot] (segment sum)
  - per window: MLP psum2[j, slot] = W^T @ cast_bf16(psum);
    relu(+b) into a multi-window batch tile; DMA out per batch.
Host scatters out[128, NWIN*WINW] per core back to z[50000, 128] via the
window/slot -> node map.
"""

import sys

if "/opt/trn_rl_repo" not in sys.path:
    sys.path.insert(0, "/opt/trn_rl_repo")

import numpy as np
import ml_dtypes

import concourse.bass as bass
import concourse.bacc as bacc
import concourse.mybir as mybir
import concourse.tile as tile
from concourse.bass_utils import run_bass_kernel_spmd

N_NODES = 50000
DIM = 128
NCORES = 8
NPC = N_NODES // NCORES          # 6250 dst nodes per core
WINW = 32                        # dst-window width (one-hot/psum free dim)
TW = 4                           # tiles per window (fixed; one STT builds all 4)
CHUNK_TILES = 64                 # steady-state tiles per hg stream chunk (2 MB)
SEG_BOUNDS = (32, 192, 640)      # coeff segment cuts (multiples of TW):
                                 # [0,30) on DVE, rest gpsimd/deferred
GROUP = 16                       # windows per PSUM bank / evacuation batch
P = 128

f32 = mybir.dt.float32
bf16 = mybir.dt.bfloat16


def _pack_partitions(deg):
    """Assign each dst node ceil(deg/TW) partitions within some window so the
    window's slot vector is constant across its TW tiles (one multi-tile STT
    builds all TW one-hots).  Windows take <=WINW dsts and <=128 partitions.
    Greedy largest-fit keeps partitions ~full.  Returns per-node (win, slot,
    p0) and the number of windows."""
    q = np.maximum((deg + TW - 1) // TW, 1).astype(np.int64)  # partitions/dst
    order = np.argsort(-deg, kind="stable")
    # pools by q, each a stack (desc degree -> pop from front via index)
    pools = {}
    for idx in order:
        pools.setdefault(int(q[idx]), []).append(int(idx))
    ptrs = {k: 0 for k in pools}
    qs_desc = sorted(pools.keys(), reverse=True)

    win_of = np.full(NPC, -1, np.int64)
    slot_of = np.zeros(NPC, np.int64)
    p0_of = np.zeros(NPC, np.int64)
    n_left = NPC
    w = 0
    while n_left > 0:
        parts = 0
        slots = 0
        while slots < WINW and parts < 128:
            qv = None
            for qc in qs_desc:
                if ptrs[qc] < len(pools[qc]) and parts + qc <= 128:
                    qv = qc
                    break
            if qv is None:
                break
            idx = pools[qv][ptrs[qv]]
            ptrs[qv] += 1
            win_of[idx] = w
            slot_of[idx] = slots
            p0_of[idx] = parts
            parts += qv
            slots += 1
            n_left -= 1
        w += 1
    return win_of, slot_of, p0_of, w


def _preprocess(h, alpha, edge_weight, W, b, node_id, edge_src, edge_dst):
    """Host-side planning: sort/pad edges, pack device images. Data movement only."""
    src = np.asarray(edge_src).astype(np.int64)
    dst = np.asarray(edge_dst).astype(np.int64)
    node_id = np.asarray(node_id).astype(np.int64)
    alpha = np.asarray(alpha, dtype=np.float32)
    ew = np.asarray(edge_weight, dtype=np.float32)
    E = src.shape[0]
    gene_num = alpha.shape[0] - 2

    src_id = node_id[src]
    dst_id = node_id[dst]
    gi = np.full(E, gene_num + 1, np.int64)
    gi = np.where((src_id >= 0) & (dst_id < 0), src_id, gi)
    gi = np.where((dst_id >= 0) & (src_id < 0), dst_id, gi)
    gi = np.where((dst_id >= 0) & (src_id >= 0), gene_num, gi)
    a_e = alpha[gi]                                   # gather (data movement)

    deg = np.bincount(dst, minlength=N_NODES).astype(np.float32)
    r_e = 1.0 / np.maximum(deg[dst], 1.0)             # mean norm (metadata)

    core = dst // NPC
    ldst = dst - core * NPC

    # per-core partition-ownership packing (planning only)
    degc = np.bincount(dst, minlength=N_NODES).astype(np.int64).reshape(NCORES, NPC)
    win_of = np.zeros((NCORES, NPC), np.int64)
    slot_of = np.zeros((NCORES, NPC), np.int64)
    p0_of = np.zeros((NCORES, NPC), np.int64)
    nw = 0
    for c in range(NCORES):
        win_of[c], slot_of[c], p0_of[c], w_c = _pack_partitions(degc[c])
        nw = max(nw, w_c)
    NWIN = nw
    TT = NWIN * TW
    EP = TT * P

    # per-edge placement: edges of dst d fill its q_d x TW grid row-major
    eorder = np.argsort(core * NPC + ldst, kind="stable")   # sort by (core,dst)
    dkey = (core * NPC + ldst)[eorder]
    first = np.ones(E, bool)
    first[1:] = dkey[1:] != dkey[:-1]
    gstart = np.where(first)[0]
    gsid = np.cumsum(first) - 1
    rank = np.arange(E, dtype=np.int64) - gstart[gsid]      # rank within dst

    ec = core[eorder]
    el = ldst[eorder]
    part = p0_of[ec, el] + rank // TW
    tl = win_of[ec, el] * TW + rank % TW
    pos = tl * P + part                                      # within core image

    gidx_p = np.zeros((NCORES, EP), np.int32)
    a_p = np.zeros((NCORES, EP), np.float32)
    w_p = np.zeros((NCORES, EP), np.float32)
    cnt_p = np.zeros((NCORES, EP), np.float32)
    gidx_p[ec, pos] = src[eorder].astype(np.int32)
    a_p[ec, pos] = a_e[eorder]
    w_p[ec, pos] = ew[eorder]
    cnt_p[ec, pos] = r_e[eorder]
    # per-tile slot image kept for the test-harness emulator only
    slot_p = np.zeros((NCORES, EP), np.float32)
    slot_p[ec, pos] = slot_of[ec, el].astype(np.float32)

    # per-(partition, window) slot vector for the device (constant over TW tiles)
    slotw = np.zeros((NCORES, P, NWIN), np.float32)
    for c in range(NCORES):
        owned = win_of[c] >= 0
        d = np.where(owned)[0]
        for di in d:
            q = max((degc[c][di] + TW - 1) // TW, 1)
            slotw[c, p0_of[c][di]:p0_of[c][di] + q, win_of[c][di]] = slot_of[c][di]

    # images: edge pos = t*128 + p  ->  [p, t]
    def img(x):
        return np.ascontiguousarray(x.reshape(NCORES, TT, P).transpose(0, 2, 1))

    a_i, w_i, cnt_i, slot_i = img(a_p), img(w_p), img(cnt_p), img(slot_p)
    # packed per-segment meta images: blocks [cnt | a | w] so ONE DMA brings a
    # whole segment; meta0 additionally carries [slotw | bias]
    cuts = [0] + [min(b, TT) for b in SEG_BOUNDS] + [TT]
    segs = [(lo, hi) for lo, hi in zip(cuts, cuts[1:]) if hi > lo]

    def meta_img(lo, hi):
        return np.ascontiguousarray(np.concatenate(
            [cnt_i[:, :, lo:hi], a_i[:, :, lo:hi], w_i[:, :, lo:hi]], axis=2))

    metas = [meta_img(lo, hi) for lo, hi in segs]
    b_img = np.broadcast_to(
        np.asarray(b, np.float32).reshape(1, DIM, 1), (NCORES, DIM, 1))
    metas[0] = np.ascontiguousarray(
        np.concatenate([metas[0], slotw, b_img], axis=2))

    h_bf = np.asarray(h, np.float32).astype(ml_dtypes.bfloat16)
    # per-core source-feature stream, laid out exactly as consumed:
    # [128 partitions, TT tiles, DIM] with edge (t, p) at [p, t, :]
    hg_img = np.ascontiguousarray(
        h_bf[gidx_p.reshape(NCORES, TT, P)].transpose(0, 2, 1, 3))

    # out column (w*WINW + s) -> global node id (or -1)
    outmap = np.full((NCORES, NWIN * WINW), -1, np.int64)
    for c in range(NCORES):
        cols = win_of[c] * WINW + slot_of[c]
        outmap[c, cols] = c * NPC + np.arange(NPC)

    T = np.full(NWIN, TW, np.int64)
    tile_off = np.arange(NWIN, dtype=np.int64) * TW
    plan = dict(
        T=T, TT=TT, EP=EP, NWIN=NWIN, tile_off=tile_off, segs=segs,
        hg_img=hg_img, metas=metas, outmap=outmap,
        # kept for test harness emulation compatibility
        slot_img=slot_i, a_img=a_i, w_img=w_i, cnt_img=cnt_i, idx_img=img(gidx_p),
        wt_bf=np.ascontiguousarray(np.asarray(W, np.float32).T).astype(ml_dtypes.bfloat16),
        b_col=np.ascontiguousarray(np.asarray(b, np.float32).reshape(DIM, 1)),
    )
    return plan


def _reassemble(plan, outs):
    """outs: per-core [128, NWIN*WINW] arrays -> z [N_NODES, DIM]."""
    z = np.empty((N_NODES, DIM), np.float32)
    outmap = plan["outmap"]
    for c in range(NCORES):
        valid = outmap[c] >= 0
        z[outmap[c][valid]] = np.asarray(outs[c])[:, valid].T
    return z


def _build(plan):
    """Build the (SPMD-identical) Bass graph from the static plan."""
    TT = plan["TT"]
    NWIN = plan["NWIN"]
    segs = plan["segs"]

    nc = bacc.Bacc("TRN2", target_bir_lowering=False, debug=False,
                   num_swdge_queues=4)
    hg_d = nc.dram_tensor("hgimg", [P, TT, DIM], bf16, kind="ExternalInput")
    meta_d = [nc.dram_tensor(f"meta{i}",
                             [P, 3 * (hi - lo) + (NWIN + 1 if i == 0 else 0)],
                             f32, kind="ExternalInput")
              for i, (lo, hi) in enumerate(segs)]
    wt_d = nc.dram_tensor("wt", [DIM, DIM], bf16, kind="ExternalInput")
    out_d = nc.dram_tensor("out", [P, NWIN * WINW], f32, kind="ExternalOutput")

    with tile.TileContext(nc) as tc:
        with (
            tc.tile_pool(name="const", bufs=1) as cpool,
            tc.tile_pool(name="gather", bufs=5) as gpool,
            tc.tile_pool(name="oh", bufs=64) as ohpool,
            tc.tile_pool(name="mlp", bufs=3) as mpool,
            tc.tile_pool(name="zb", bufs=3) as zpool,
            tc.tile_pool(name="psum", bufs=2, space="PSUM") as pspool,
            tc.tile_pool(name="psum2", bufs=2, space="PSUM") as ps2pool,
        ):
            iota_f = cpool.tile([P, WINW], f32, tag="iotaf")
            nc.gpsimd.iota(iota_f[:], pattern=[[1, WINW]], base=0,
                           channel_multiplier=0,
                           allow_small_or_imprecise_dtypes=True)
            iotar = cpool.tile([P, TW * WINW], bf16, tag="iotar")
            for j in range(TW):
                nc.vector.tensor_copy(out=iotar[:, j * WINW:(j + 1) * WINW],
                                      in_=iota_f[:])

            # coeff = a*w*(1/cnt) per segment.  Segment 0 on DVE via the
            # scalar (ACT) HWDGE ring (fast pipeline head); segments 1-2 on
            # gpsimd (own SWDGE queue) so the DVE one-hot stream and the
            # sync ring (hg chunks) stay clear.
            meta_sb, coeff_sb = [], []
            for i, (lo, hi) in enumerate(segs):
                n = hi - lo
                m = cpool.tile([P, 3 * n + (NWIN + 1 if i == 0 else 0)], f32,
                               tag=f"meta{i}", name="meta")
                c_ = cpool.tile([P, n], f32, tag=f"coeff{i}", name="coeff")
                meta_sb.append(m)
                coeff_sb.append(c_)

            def emit_seg(i):
                # segs 0-1: ACT-ring DMA + DVE products (fast pipeline head);
                # later segs: gpsimd products, last seg's DMA deferred into
                # the ACT engine's program order (emitted at a later window)
                # to keep its bytes out of the congested early HBM window
                lo, hi = segs[i]
                n = hi - lo
                m, c_ = meta_sb[i], coeff_sb[i]
                if i == 0:
                    nc.scalar.dma_start(m[:], meta_d[i].ap()[:])
                    eng = nc.vector
                elif i >= 2:
                    # deferred: DMA rides the ACT ring at a later window so
                    # its bytes stay out of the HBM-saturated early phase
                    nc.scalar.dma_start(m[:], meta_d[i].ap()[:])
                    eng = nc.gpsimd
                else:
                    nc.gpsimd.dma_start(m[:], meta_d[i].ap()[:])
                    eng = nc.gpsimd
                eng.tensor_tensor(out=c_[:], in0=m[:, n:2 * n],
                                  in1=m[:, 2 * n:3 * n],
                                  op=mybir.AluOpType.mult)
                eng.tensor_tensor(out=c_[:], in0=c_[:], in1=m[:, 0:n],
                                  op=mybir.AluOpType.mult)

            emit_seg(0)

            def seg_of(t):
                for i, (lo, hi) in enumerate(segs):
                    if t < hi:
                        return i, t - lo
                raise AssertionError

            n0 = segs[0][1] - segs[0][0]

            def slotw_ap(w):
                return meta_sb[0][:, 3 * n0 + w: 3 * n0 + w + 1]

            def coeff_run(t0):
                # [P, TW] view of the coeff columns for tiles t0..t0+TW-1
                # (SEG_BOUNDS are multiples of TW so a window never straddles)
                i, k = seg_of(t0)
                return coeff_sb[i][:, k: k + TW]

            wt_sb = cpool.tile([DIM, DIM], bf16, tag="wt")
            nc.sync.dma_start(wt_sb[:], wt_d.ap()[:])
            b_sb = meta_sb[0]

            # ramped chunk plan: small head chunks land fast even while the
            # meta/weight transfers share the SDMA engines, then 64s
            chunks = []
            t0c = 0
            for first in (4, 8, 16, 32):
                if t0c < TT:
                    nt = min(first, TT - t0c)
                    chunks.append((t0c, nt))
                    t0c += nt
            while t0c < TT:
                nt = min(CHUNK_TILES, TT - t0c)
                chunks.append((t0c, nt))
                t0c += nt
            tile2chunk = {}
            for ci, (c0, nt) in enumerate(chunks):
                for k in range(nt):
                    tile2chunk[c0 + k] = (ci, k)

            stream_tiles = {}

            def ensure_streamed(ci):
                if ci in stream_tiles:
                    return stream_tiles[ci]
                c0, nt = chunks[ci]
                hg = gpool.tile([P, CHUNK_TILES, DIM], bf16, tag="hg", name="hg")
                nc.sync.dma_start(hg[:, :nt, :], hg_d.ap()[:, c0:c0 + nt, :])
                stream_tiles[ci] = hg
                return hg

            if len(segs) > 1:
                emit_seg(1)
            # segs 2+ emitted inside the window loop (window 15, 40, ...)
            defer_at = {15 + 25 * (i - 2): i for i in range(2, len(segs))}

            # GROUP windows share one PSUM bank (GROUP*WINW <= 512 f32);
            # start=True only on the group's first matmul: it clears the
            # whole bank's has_written bits, each later window's first MM
            # then overwrites its own (cleared) column region and subsequent
            # MMs accumulate -- one ACT copy + one N=GROUP*WINW MLP matmul
            # + one relu + one out-DMA per GROUP instead of per window.
            psumG = None
            for w in range(NWIN):
                if w in defer_at:
                    emit_seg(defer_at.pop(w))
                t0 = w * TW
                g = w % GROUP
                if g == 0:
                    psumG = pspool.tile([P, GROUP * WINW], f32, tag="ps",
                                        name="psum")
                last_of_group = (g == GROUP - 1) or (w == NWIN - 1)
                # ONE multi-tile STT builds all TW one-hots of the window:
                # out[p, j, s] = (iota[s] == slotw[p]) * coeff[p, t0+j]
                # (the window's slot vector is constant across its tiles)
                oh5 = ohpool.tile([P, TW * WINW], bf16, tag="oh", name="oh")
                nc.vector.scalar_tensor_tensor(
                    out=oh5[:].rearrange("p (k s) -> p k s", k=TW),
                    in0=iotar[:].rearrange("p (k s) -> p k s", k=TW),
                    scalar=slotw_ap(w),
                    in1=coeff_run(t0).unsqueeze(2).broadcast_to([P, TW, WINW]),
                    op0=mybir.AluOpType.is_equal,
                    op1=mybir.AluOpType.mult,
                )
                for k in range(TW):
                    t = t0 + k
                    ci, kk = tile2chunk[t]
                    hg = ensure_streamed(ci)
                    nc.tensor.matmul(
                        psumG[:, g * WINW:(g + 1) * WINW], hg[:, kk, :],
                        oh5[:, k * WINW:(k + 1) * WINW],
                        start=(k == 0), stop=(k == TW - 1),
                    )
                if last_of_group:
                    gw = (g + 1) * WINW
                    w0 = w - g
                    nbf = mpool.tile([P, GROUP * WINW], bf16, tag="nbf",
                                     name="nbf")
                    nc.scalar.copy(nbf[:, :gw], psumG[:, :gw])
                    psum2 = ps2pool.tile([P, GROUP * WINW], f32, tag="ps2",
                                         name="psum2")
                    nc.tensor.matmul(psum2[:, :gw], wt_sb[:], nbf[:, :gw],
                                     start=True, stop=True)
                    zb = zpool.tile([P, GROUP * WINW], f32, tag="zb",
                                    name="zbat")
                    nc.scalar.activation(zb[:, :gw], psum2[:, :gw],
                                         mybir.ActivationFunctionType.Relu,
                                         bias=b_sb[:, 3 * n0 + NWIN:
                                                   3 * n0 + NWIN + 1])
                    nc.scalar.dma_start(
                        out_d.ap()[:, w0 * WINW: w0 * WINW + gw],
                        zb[:, :gw])

    nc.compile()
    return nc


def _in_maps(plan):
    maps = []
    for c in range(NCORES):
        m = {
            "hgimg": plan["hg_img"][c],
            "wt": plan["wt_bf"],
        }
        for i in range(len(plan["segs"])):
            m[f"meta{i}"] = plan["metas"][i][c]
        maps.append(m)
    return maps


_NC_CACHE = {}


def _get_nc(plan):
    key = (plan["TT"], tuple(plan["T"]))
    if key not in _NC_CACHE:
        _NC_CACHE[key] = _build(plan)
    return _NC_CACHE[key]


def kernel(**inputs):
    plan = _preprocess(**{k: np.asarray(v) for k, v in inputs.items()})
    nc = _get_nc(plan)
    res = run_bass_kernel_spmd(nc, _in_maps(plan), core_ids=list(range(NCORES)))
    return _reassemble(plan, [res.results[c]["out"] for c in range(NCORES)])
